# revision 17
# baseline (speedup 1.0000x reference)
"""GNN message passing (edge-conv + segment-max) on 8 Trainium2 cores.

Sharding: edges partitioned by destination node range (core c owns dst nodes
[c*6250, (c+1)*6250)), so segment-max aggregation is fully core-local.

Layout ("stacked halo-ELL"): per core, each dst node with degree d gets
ceil(d/K) columns (K=18 rank cap; extra "fold" columns are max-merged on the
host after the final tanh, which commutes with max). Columns are
degree-sorted and interleaved into two halves (even sorted-rank ->
partitions 0:64, odd -> 64:128); each half gets an ELL rank-row layout
(rank k covers the dense prefix of columns with column-degree > k). The
host materializes the source halo: x_exp[:, slot] = x[src(slot)] in fp16,
stacked [128, TOT], so the device does NO gathers at all (bypassing the
~8ns/idx GPSIMD SWDGE descriptor-generation bottleneck; everything streams
via HWDGE DMA).

Device pipeline per 1024-slot chunk (all ops full 128 partitions):
  dma x_exp [128,1024] -> PE: ph = blkdiag(W1b^T).T @ x_exp
                               + blkdiag((W1a-W1b)^T).T @ xiT_cols (2x N=512)
  -> ACT: h = LeakyReLU(ph + b1) fp16 -> PE: msg = blkdiag(W2^T).T @ h
  -> DVE: A = max(A, msg).
Chunks are emitted rank-major with a per-rank rotation so consecutive chunks
touch disjoint A column ranges. Final tanh(A + b2) fused on ACT; host
un-stacks, merges fold columns, applies the empty-segment 0 fill.
"""

import numpy as np

import concourse.bacc as bacc
import concourse.mybir as mybir
import concourse.tile as tile
from concourse.bass_utils import run_bass_kernel_spmd

F16 = np.dtype(np.float16)

N_NODES = 50000
N_EDGES = 800000
D = 64
NC = 8
NPC = N_NODES // NC          # 6250 dst nodes per core
P = 128
LEAKY = 0.01
NEG_INIT = -1.0e30
MM = 512                     # matmul free dim (PSUM bank limit)
PAIR = 2 * MM                # compute chunk width
FOLD_K = 12                  # ELL rank cap (deeper edges fold to new columns)

_CACHE = {}


def _roundup(a, m):
    return (a + m - 1) // m * m


def _build_program(w_list, xw):
    nc = bacc.Bacc("TRN2", target_bir_lowering=False, debug=False, num_devices=NC)
    dt = mybir.dt
    tot = int(sum(w_list))
    xexp = nc.dram_tensor("xexp", [P, tot], dt.float16, kind="ExternalInput")
    xiT = nc.dram_tensor("xiT", [P, xw], dt.float16, kind="ExternalInput")
    w3_blk = nc.dram_tensor("w3_blk", [P, 3 * P], dt.float16, kind="ExternalInput")
    b12 = nc.dram_tensor("b12", [P, 2], dt.float32, kind="ExternalInput")
    outs = [nc.dram_tensor(f"out{s}", [P, MM], dt.float32, kind="ExternalOutput")
            for s in range(xw // MM)]

    # chunk emission order: rank-major, rotated within each rank so that
    # consecutive chunks (and rank-boundary neighbors) touch different
    # A column ranges.
    chunks = []
    tails = []
    goff = 0
    for k, wk in enumerate(w_list):
        cl = [(goff + c0, c0, min(PAIR, wk - c0), k == 0)
              for c0 in range(0, wk, PAIR)]
        if k > 0 and cl and cl[-1][2] < PAIR:
            tails.append(cl.pop())
        n = len(cl)
        rot = (k * (n // 2 + 1)) % n if n > 1 else 0
        chunks.extend(cl[rot:] + cl[:rot])
        goff += wk
    # small per-rank tails last, so the main stream is uniform 1024-wide;
    # interleave tails from distant ranks to space column conflicts.
    # rank-0 chunks stay in front (and its tail in the main stream): they
    # initialize A by plain copy, so they must precede every max to their
    # columns.
    tails.sort(key=lambda t: t[1])
    chunks.extend(tails[0::2] + tails[1::2])

    with tile.TileContext(nc) as tc:
        with (
            tc.tile_pool(name="const", bufs=1) as cpool,
            tc.tile_pool(name="xin", bufs=10) as xpool,
            tc.tile_pool(name="hbuf", bufs=6) as hpool,
            tc.tile_pool(name="psA", bufs=2, space="PSUM") as pApool,
            tc.tile_pool(name="psB", bufs=2, space="PSUM") as pBpool,
        ):
            w3_sb = cpool.tile([P, 3 * P], dt.float16, tag="w3")
            nc.sync.dma_start(out=w3_sb[:], in_=w3_blk[:, :])
            wb_sb = w3_sb[:, 0:P]
            wa_sb = w3_sb[:, P : 2 * P]
            w2_sb = w3_sb[:, 2 * P : 3 * P]
            b12_sb = cpool.tile([P, 2], dt.float32, tag="b12")
            nc.sync.dma_start(out=b12_sb[:], in_=b12[:, :])
            b1_sb = b12_sb[:, 0:1]
            b2_sb = b12_sb[:, 1:2]
            # first chunk's x_exp before the (big) xiT transfer
            xc0 = xpool.tile([P, PAIR], dt.float16, tag="xc")
            gs0, _, w0, _ = chunks[0]
            nc.sync.dma_start(out=xc0[:, 0:w0], in_=xexp[:, gs0 : gs0 + w0])
            xiT_sb = cpool.tile([P, xw], dt.float16, tag="xiT")
            for q0 in range(0, xw, 896):
                q1 = min(q0 + 896, xw)
                nc.sync.dma_start(out=xiT_sb[:, q0:q1], in_=xiT[:, q0:q1])
            A = cpool.tile([P, xw], dt.float32, tag="A")

            def emit_tail(h, c0, w, first):
                # W2 matmul + max for a chunk whose LReLU was already issued;
                # deferred one chunk so the PE never stalls waiting on ACT.
                # rank-0 chunks initialize A by copy (no memset needed).
                pm = pBpool.tile([P, PAIR], dt.float32, tag="pm")
                for o in range(0, w, MM):
                    m = min(MM, w - o)
                    nc.tensor.matmul(
                        out=pm[:, o : o + m], lhsT=w2_sb, rhs=h[:, o : o + m],
                        start=True, stop=True,
                    )
                if first:
                    nc.vector.tensor_copy(
                        out=A[:, c0 : c0 + w], in_=pm[:, 0:w],
                    )
                else:
                    nc.vector.tensor_tensor(
                        out=A[:, c0 : c0 + w], in0=A[:, c0 : c0 + w],
                        in1=pm[:, 0:w], op=mybir.AluOpType.max,
                    )

            # per-stripe last-touching chunk (by emission index)
            nstripes = xw // MM
            last_touch = [0] * nstripes
            for ci, (gs, c0, w, first) in enumerate(chunks):
                for s in range(c0 // MM, min((c0 + w - 1) // MM + 1, nstripes)):
                    last_touch[s] = ci
            stripes_after = {}
            nch = len(chunks)
            for s, ci in enumerate(last_touch):
                # delay a few chunks so the in-order Scalar queue never
                # stalls on the stripe's pending A-updates
                stripes_after.setdefault(min(ci + 3, nch - 1), []).append(s)

            def emit_stripe(s):
                s0 = s * MM
                fin = hpool.tile([P, PAIR], dt.float32, tag="fin")
                nc.scalar.activation(
                    out=fin[:, 0:MM], in_=A[:, s0 : s0 + MM],
                    func=mybir.ActivationFunctionType.Tanh,
                    bias=b2_sb,
                )
                nc.sync.dma_start(out=outs[s][:, :], in_=fin[:, 0:MM])

            pending = None
            pending_stripes = None
            for ci, (gs, c0, w, first) in enumerate(chunks):
                if ci == 0:
                    xc = xc0
                else:
                    xc = xpool.tile([P, PAIR], dt.float16, tag="xc")
                    nc.sync.dma_start(out=xc[:, 0:w], in_=xexp[:, gs : gs + w])
                ph = pApool.tile([P, PAIR], dt.float32, tag="ph")
                for o in range(0, w, MM):
                    m = min(MM, w - o)
                    nc.tensor.matmul(
                        out=ph[:, o : o + m], lhsT=wb_sb, rhs=xc[:, o : o + m],
                        start=True, stop=False,
                    )
                for o in range(0, w, MM):
                    m = min(MM, w - o)
                    nc.tensor.matmul(
                        out=ph[:, o : o + m], lhsT=wa_sb,
                        rhs=xiT_sb[:, c0 + o : c0 + o + m],
                        start=False, stop=True,
                    )
                if pending is not None:
                    emit_tail(*pending)
                    if pending_stripes:
                        for s in pending_stripes:
                            emit_stripe(s)
                h = hpool.tile([P, PAIR], dt.float16, tag="h")
                nc.scalar.activation(
                    out=h[:, 0:w], in_=ph[:, 0:w],
                    func=mybir.ActivationFunctionType.Lrelu,
                    bias=b1_sb, alpha=LEAKY,
                )
                pending = (h, c0, w, first)
                pending_stripes = stripes_after.get(ci)
            emit_tail(*pending)
            if pending_stripes:
                for s in pending_stripes:
                    emit_stripe(s)
    nc.compile()
    return nc


def _host_prep(x, edge_index, W1, b1, W2, b2):
    src = np.asarray(edge_index[0], dtype=np.int64)
    dst = np.asarray(edge_index[1], dtype=np.int64)
    x = np.ascontiguousarray(np.asarray(x, dtype=np.float32))
    x_f16 = x.astype(F16)

    W1 = np.asarray(W1, dtype=np.float64)
    W2 = np.asarray(W2, dtype=np.float64)
    W1a, W1b = W1[:, :D], W1[:, D:]
    Wa = W1a - W1b

    def blk(M):
        Z = np.zeros((P, P), np.float64)
        Z[:D, :D] = M.T
        Z[D:, D:] = M.T
        return np.ascontiguousarray(Z).astype(F16)

    w3_blk = np.concatenate([blk(W1b), blk(Wa), blk(np.asarray(W2))], axis=1)
    b12 = np.stack([np.tile(np.asarray(b1, np.float32), 2),
                    np.tile(np.asarray(b2, np.float32), 2)], axis=1).astype(np.float32)

    per_core = []
    for c in range(NC):
        sel = (dst // NPC) == c
        s_c = src[sel]
        d_c = dst[sel] - c * NPC
        deg = np.bincount(d_c, minlength=NPC)
        order = np.argsort(d_c, kind="stable")
        ds = d_c[order]          # local dst per edge (dst-sorted)
        ss = s_c[order]          # src per edge
        starts = np.zeros(NPC + 1, np.int64)
        starts[1:] = np.cumsum(deg)
        erank = np.arange(len(ds), dtype=np.int64) - starts[ds]
        # fold: edge -> (column id, rank)
        sub = erank // FOLD_K    # sub-column index within node
        crank = erank % FOLD_K   # rank within column
        # columns: (node n, sub s) for s < ceil(deg/K); column degree:
        ncols_node = (deg + FOLD_K - 1) // FOLD_K  # 0 for deg=0
        col_off = np.zeros(NPC + 1, np.int64)
        col_off[1:] = np.cumsum(ncols_node)
        ncol = int(col_off[-1])
        col_id = col_off[ds] + sub               # per edge
        # per-column node and degree
        col_node = np.repeat(np.arange(NPC), ncols_node)
        col_sub = np.arange(ncol) - col_off[col_node]
        col_deg = np.minimum(deg[col_node] - col_sub * FOLD_K, FOLD_K)
        per_core.append(dict(
            deg=deg, ds=ds, ss=ss, starts=starts, crank=crank,
            col_id=col_id, col_node=col_node, col_deg=col_deg, ncol=ncol,
        ))

    max_ncol = max(pc["ncol"] for pc in per_core)
    xw = _roundup((max_ncol + 1) // 2, MM)

    # per-rank per-half padded widths (common across cores)
    w_list = []
    for k in range(FOLD_K):
        n_k = 0
        for pc in per_core:
            cnt = int((pc["col_deg"] > k).sum())
            n_k = max(n_k, (cnt + 1) // 2)
        w_list.append(max(P, _roundup(n_k, P)))
    offs = np.concatenate([[0], np.cumsum(w_list)]).astype(np.int64)
    tot = int(offs[-1])

    in_maps = []
    metas = []
    for c in range(NC):
        pc = per_core[c]
        ncol = pc["ncol"]
        # sort columns by degree desc (stable), interleave halves
        csort = np.argsort(-pc["col_deg"], kind="stable")   # sorted pos -> col
        srank = np.empty(ncol, np.int64)
        srank[csort] = np.arange(ncol)
        half = srank % 2
        pos = srank // 2

        first_src = np.zeros(ncol, np.int64)
        # rank-0 edge of each column: edges with crank==0
        m0 = pc["crank"] == 0
        first_src[pc["col_id"][m0]] = pc["ss"][m0]

        hp_node = np.zeros((2, xw), np.int64)    # node of column at (half,pos)
        hp_src = np.zeros((2, xw), np.int64)     # dup src for pad slots
        hp_node[half, pos] = pc["col_node"]
        hp_src[half, pos] = first_src

        src_slot = np.empty((2, tot), np.int64)
        for k in range(FOLD_K):
            src_slot[:, offs[k] : offs[k + 1]] = hp_src[:, : w_list[k]]
        src_slot[half[pc["col_id"]], offs[pc["crank"]] + pos[pc["col_id"]]] = pc["ss"]

        xexp = np.empty((P, tot), F16)
        xexp[0:D, :] = x_f16[src_slot[0]].T
        xexp[D:P, :] = x_f16[src_slot[1]].T

        xiT = np.zeros((P, xw), F16)
        xiT[0:D, :] = x_f16[c * NPC + hp_node[0]].T
        xiT[D:P, :] = x_f16[c * NPC + hp_node[1]].T

        in_maps.append({
            "xexp": xexp, "xiT": xiT,
            "w3_blk": w3_blk, "b12": b12,
        })
        metas.append(dict(half=half, pos=pos, col_node=pc["col_node"],
                          deg=pc["deg"], ncol=ncol))

    meta = dict(w_list=tuple(int(w) for w in w_list), xw=xw, metas=metas)
    return in_maps, meta


def _run(inputs, trace=False):
    in_maps, meta = _host_prep(
        inputs["x"], inputs["edge_index"], inputs["W1"], inputs["b1"],
        inputs["W2"], inputs["b2"],
    )
    key = (meta["w_list"], meta["xw"])
    if key not in _CACHE:
        _CACHE[key] = _build_program(list(meta["w_list"]), meta["xw"])
    nc = _CACHE[key]
    res = run_bass_kernel_spmd(nc, in_maps, core_ids=list(range(NC)), trace=trace)

    out = np.full((N_NODES, D), -np.inf, np.float32)
    for c in range(NC):
        mc = meta["metas"][c]
        rr = res.results[c]
        r = np.concatenate([rr[f"out{s}"] for s in range(meta["xw"] // MM)], axis=1)
        half, pos, col_node = mc["half"], mc["pos"], mc["col_node"]
        vals = np.empty((mc["ncol"], D), np.float32)
        h0 = half == 0
        vals[h0] = r[0:D, :][:, pos[h0]].T
        vals[~h0] = r[D:P, :][:, pos[~h0]].T
        # merge fold columns per node (max; tanh is monotone)
        nodes = c * NPC + col_node
        np.maximum.at(out, nodes, vals)
        out[c * NPC + np.arange(NPC)[mc["deg"] == 0]] = 0.0
    out[~np.isfinite(out)] = 0.0
    return out, res


def kernel(**inputs) -> np.ndarray:
    out, _ = _run(inputs, trace=False)
    return out


# revision 18
# speedup vs baseline: 1.0278x; 1.0278x over previous
"""GNN message passing (edge-conv + segment-max) on 8 Trainium2 cores.

Sharding: edges partitioned by destination node range (core c owns dst nodes
[c*6250, (c+1)*6250)), so segment-max aggregation is fully core-local.

Layout ("stacked halo-ELL"): per core, each dst node with degree d gets
ceil(d/K) columns (K=18 rank cap; extra "fold" columns are max-merged on the
host after the final tanh, which commutes with max). Columns are
degree-sorted and interleaved into two halves (even sorted-rank ->
partitions 0:64, odd -> 64:128); each half gets an ELL rank-row layout
(rank k covers the dense prefix of columns with column-degree > k). The
host materializes the source halo: x_exp[:, slot] = x[src(slot)] in fp16,
stacked [128, TOT], so the device does NO gathers at all (bypassing the
~8ns/idx GPSIMD SWDGE descriptor-generation bottleneck; everything streams
via HWDGE DMA).

Device pipeline per 1024-slot chunk (all ops full 128 partitions):
  dma x_exp [128,1024] -> PE: ph = blkdiag(W1b^T).T @ x_exp
                               + blkdiag((W1a-W1b)^T).T @ xiT_cols (2x N=512)
  -> ACT: h = LeakyReLU(ph + b1) fp16 -> PE: msg = blkdiag(W2^T).T @ h
  -> DVE: A = max(A, msg).
Chunks are emitted rank-major with a per-rank rotation so consecutive chunks
touch disjoint A column ranges. Final tanh(A + b2) fused on ACT; host
un-stacks, merges fold columns, applies the empty-segment 0 fill.
"""

import numpy as np

import concourse.bacc as bacc
import concourse.mybir as mybir
import concourse.tile as tile
from concourse.bass_utils import run_bass_kernel_spmd

F16 = np.dtype(np.float16)

N_NODES = 50000
N_EDGES = 800000
D = 64
NC = 8
NPC = N_NODES // NC          # 6250 dst nodes per core
P = 128
LEAKY = 0.01
NEG_INIT = -1.0e30
MM = 512                     # matmul free dim (PSUM bank limit)
PAIR = 2 * MM                # compute chunk width
FOLD_K = 18                  # ELL rank cap (deeper edges fold to new columns)

_CACHE = {}


def _roundup(a, m):
    return (a + m - 1) // m * m


def _build_program(w_list, xw):
    nc = bacc.Bacc("TRN2", target_bir_lowering=False, debug=False, num_devices=NC)
    dt = mybir.dt
    tot = int(sum(w_list))
    xexp = nc.dram_tensor("xexp", [P, tot], dt.float16, kind="ExternalInput")
    xiT = nc.dram_tensor("xiT", [P, xw], dt.float16, kind="ExternalInput")
    w3_blk = nc.dram_tensor("w3_blk", [P, 3 * P], dt.float16, kind="ExternalInput")
    b12 = nc.dram_tensor("b12", [P, 2], dt.float32, kind="ExternalInput")
    outs = [nc.dram_tensor(f"out{s}", [P, MM], dt.float32, kind="ExternalOutput")
            for s in range(xw // MM)]

    # chunk emission order: rank-major, rotated within each rank so that
    # consecutive chunks (and rank-boundary neighbors) touch different
    # A column ranges.
    chunks = []
    tails = []
    goff = 0
    for k, wk in enumerate(w_list):
        cl = [(goff + c0, c0, min(PAIR, wk - c0), k == 0)
              for c0 in range(0, wk, PAIR)]
        if k > 0 and cl and cl[-1][2] < PAIR:
            tails.append(cl.pop())
        n = len(cl)
        rot = (k * (n // 2 + 1)) % n if n > 1 else 0
        chunks.extend(cl[rot:] + cl[:rot])
        goff += wk
    # small per-rank tails last, so the main stream is uniform 1024-wide;
    # interleave tails from distant ranks to space column conflicts.
    # rank-0 chunks stay in front (and its tail in the main stream): they
    # initialize A by plain copy, so they must precede every max to their
    # columns.
    tails.sort(key=lambda t: t[1])
    chunks.extend(tails[0::2] + tails[1::2])

    with tile.TileContext(nc) as tc:
        with (
            tc.tile_pool(name="const", bufs=1) as cpool,
            tc.tile_pool(name="xin", bufs=10) as xpool,
            tc.tile_pool(name="hbuf", bufs=6) as hpool,
            tc.tile_pool(name="psA", bufs=2, space="PSUM") as pApool,
            tc.tile_pool(name="psB", bufs=2, space="PSUM") as pBpool,
        ):
            w3_sb = cpool.tile([P, 3 * P], dt.float16, tag="w3")
            nc.sync.dma_start(out=w3_sb[:], in_=w3_blk[:, :])
            wb_sb = w3_sb[:, 0:P]
            wa_sb = w3_sb[:, P : 2 * P]
            w2_sb = w3_sb[:, 2 * P : 3 * P]
            b12_sb = cpool.tile([P, 2], dt.float32, tag="b12")
            nc.sync.dma_start(out=b12_sb[:], in_=b12[:, :])
            b1_sb = b12_sb[:, 0:1]
            b2_sb = b12_sb[:, 1:2]
            # first chunk's x_exp before the (big) xiT transfer
            xc0 = xpool.tile([P, PAIR], dt.float16, tag="xc")
            gs0, _, w0, _ = chunks[0]
            nc.sync.dma_start(out=xc0[:, 0:w0], in_=xexp[:, gs0 : gs0 + w0])
            xiT_sb = cpool.tile([P, xw], dt.float16, tag="xiT")
            for q0 in range(0, xw, 896):
                q1 = min(q0 + 896, xw)
                nc.sync.dma_start(out=xiT_sb[:, q0:q1], in_=xiT[:, q0:q1])
            A = cpool.tile([P, xw], dt.float32, tag="A")

            def emit_tail(h, c0, w, first):
                # W2 matmul + max for a chunk whose LReLU was already issued;
                # deferred one chunk so the PE never stalls waiting on ACT.
                # rank-0 chunks initialize A by copy (no memset needed).
                pm = pBpool.tile([P, PAIR], dt.float32, tag="pm")
                for o in range(0, w, MM):
                    m = min(MM, w - o)
                    nc.tensor.matmul(
                        out=pm[:, o : o + m], lhsT=w2_sb, rhs=h[:, o : o + m],
                        start=True, stop=True,
                    )
                if first:
                    nc.vector.tensor_copy(
                        out=A[:, c0 : c0 + w], in_=pm[:, 0:w],
                    )
                else:
                    nc.vector.tensor_tensor(
                        out=A[:, c0 : c0 + w], in0=A[:, c0 : c0 + w],
                        in1=pm[:, 0:w], op=mybir.AluOpType.max,
                    )

            # per-stripe last-touching chunk (by emission index)
            nstripes = xw // MM
            last_touch = [0] * nstripes
            for ci, (gs, c0, w, first) in enumerate(chunks):
                for s in range(c0 // MM, min((c0 + w - 1) // MM + 1, nstripes)):
                    last_touch[s] = ci
            stripes_after = {}
            nch = len(chunks)
            for s, ci in enumerate(last_touch):
                # delay a few chunks so the in-order Scalar queue never
                # stalls on the stripe's pending A-updates
                stripes_after.setdefault(min(ci + 3, nch - 1), []).append(s)

            def emit_stripe(s):
                s0 = s * MM
                fin = hpool.tile([P, PAIR], dt.float32, tag="fin")
                nc.scalar.activation(
                    out=fin[:, 0:MM], in_=A[:, s0 : s0 + MM],
                    func=mybir.ActivationFunctionType.Tanh,
                    bias=b2_sb,
                )
                nc.sync.dma_start(out=outs[s][:, :], in_=fin[:, 0:MM])

            pending = None
            pending_stripes = None
            for ci, (gs, c0, w, first) in enumerate(chunks):
                if ci == 0:
                    xc = xc0
                else:
                    xc = xpool.tile([P, PAIR], dt.float16, tag="xc")
                    nc.sync.dma_start(out=xc[:, 0:w], in_=xexp[:, gs : gs + w])
                ph = pApool.tile([P, PAIR], dt.float32, tag="ph")
                for o in range(0, w, MM):
                    m = min(MM, w - o)
                    nc.tensor.matmul(
                        out=ph[:, o : o + m], lhsT=wb_sb, rhs=xc[:, o : o + m],
                        start=True, stop=False,
                    )
                for o in range(0, w, MM):
                    m = min(MM, w - o)
                    nc.tensor.matmul(
                        out=ph[:, o : o + m], lhsT=wa_sb,
                        rhs=xiT_sb[:, c0 + o : c0 + o + m],
                        start=False, stop=True,
                    )
                if pending is not None:
                    emit_tail(*pending)
                    if pending_stripes:
                        for s in pending_stripes:
                            emit_stripe(s)
                h = hpool.tile([P, PAIR], dt.float16, tag="h")
                nc.scalar.activation(
                    out=h[:, 0:w], in_=ph[:, 0:w],
                    func=mybir.ActivationFunctionType.Lrelu,
                    bias=b1_sb, alpha=LEAKY,
                )
                pending = (h, c0, w, first)
                pending_stripes = stripes_after.get(ci)
            emit_tail(*pending)
            if pending_stripes:
                for s in pending_stripes:
                    emit_stripe(s)
    nc.compile()
    return nc


def _host_prep(x, edge_index, W1, b1, W2, b2):
    src = np.asarray(edge_index[0], dtype=np.int64)
    dst = np.asarray(edge_index[1], dtype=np.int64)
    x = np.ascontiguousarray(np.asarray(x, dtype=np.float32))
    x_f16 = x.astype(F16)

    W1 = np.asarray(W1, dtype=np.float64)
    W2 = np.asarray(W2, dtype=np.float64)
    W1a, W1b = W1[:, :D], W1[:, D:]
    Wa = W1a - W1b

    def blk(M):
        Z = np.zeros((P, P), np.float64)
        Z[:D, :D] = M.T
        Z[D:, D:] = M.T
        return np.ascontiguousarray(Z).astype(F16)

    w3_blk = np.concatenate([blk(W1b), blk(Wa), blk(np.asarray(W2))], axis=1)
    b12 = np.stack([np.tile(np.asarray(b1, np.float32), 2),
                    np.tile(np.asarray(b2, np.float32), 2)], axis=1).astype(np.float32)

    per_core = []
    for c in range(NC):
        sel = (dst // NPC) == c
        s_c = src[sel]
        d_c = dst[sel] - c * NPC
        deg = np.bincount(d_c, minlength=NPC)
        order = np.argsort(d_c, kind="stable")
        ds = d_c[order]          # local dst per edge (dst-sorted)
        ss = s_c[order]          # src per edge
        starts = np.zeros(NPC + 1, np.int64)
        starts[1:] = np.cumsum(deg)
        erank = np.arange(len(ds), dtype=np.int64) - starts[ds]
        # fold: edge -> (column id, rank)
        sub = erank // FOLD_K    # sub-column index within node
        crank = erank % FOLD_K   # rank within column
        # columns: (node n, sub s) for s < ceil(deg/K); column degree:
        ncols_node = (deg + FOLD_K - 1) // FOLD_K  # 0 for deg=0
        col_off = np.zeros(NPC + 1, np.int64)
        col_off[1:] = np.cumsum(ncols_node)
        ncol = int(col_off[-1])
        col_id = col_off[ds] + sub               # per edge
        # per-column node and degree
        col_node = np.repeat(np.arange(NPC), ncols_node)
        col_sub = np.arange(ncol) - col_off[col_node]
        col_deg = np.minimum(deg[col_node] - col_sub * FOLD_K, FOLD_K)
        per_core.append(dict(
            deg=deg, ds=ds, ss=ss, starts=starts, crank=crank,
            col_id=col_id, col_node=col_node, col_deg=col_deg, ncol=ncol,
        ))

    max_ncol = max(pc["ncol"] for pc in per_core)
    xw = _roundup((max_ncol + 1) // 2, MM)

    # per-rank per-half padded widths (common across cores)
    w_list = []
    for k in range(FOLD_K):
        n_k = 0
        for pc in per_core:
            cnt = int((pc["col_deg"] > k).sum())
            n_k = max(n_k, (cnt + 1) // 2)
        w_list.append(max(P, _roundup(n_k, P)))
    offs = np.concatenate([[0], np.cumsum(w_list)]).astype(np.int64)
    tot = int(offs[-1])

    in_maps = []
    metas = []
    for c in range(NC):
        pc = per_core[c]
        ncol = pc["ncol"]
        # sort columns by degree desc (stable), interleave halves
        csort = np.argsort(-pc["col_deg"], kind="stable")   # sorted pos -> col
        srank = np.empty(ncol, np.int64)
        srank[csort] = np.arange(ncol)
        half = srank % 2
        pos = srank // 2

        first_src = np.zeros(ncol, np.int64)
        # rank-0 edge of each column: edges with crank==0
        m0 = pc["crank"] == 0
        first_src[pc["col_id"][m0]] = pc["ss"][m0]

        hp_node = np.zeros((2, xw), np.int64)    # node of column at (half,pos)
        hp_src = np.zeros((2, xw), np.int64)     # dup src for pad slots
        hp_node[half, pos] = pc["col_node"]
        hp_src[half, pos] = first_src

        src_slot = np.empty((2, tot), np.int64)
        for k in range(FOLD_K):
            src_slot[:, offs[k] : offs[k + 1]] = hp_src[:, : w_list[k]]
        src_slot[half[pc["col_id"]], offs[pc["crank"]] + pos[pc["col_id"]]] = pc["ss"]

        xexp = np.empty((P, tot), F16)
        xexp[0:D, :] = x_f16[src_slot[0]].T
        xexp[D:P, :] = x_f16[src_slot[1]].T

        xiT = np.zeros((P, xw), F16)
        xiT[0:D, :] = x_f16[c * NPC + hp_node[0]].T
        xiT[D:P, :] = x_f16[c * NPC + hp_node[1]].T

        in_maps.append({
            "xexp": xexp, "xiT": xiT,
            "w3_blk": w3_blk, "b12": b12,
        })
        metas.append(dict(half=half, pos=pos, col_node=pc["col_node"],
                          deg=pc["deg"], ncol=ncol))

    meta = dict(w_list=tuple(int(w) for w in w_list), xw=xw, metas=metas)
    return in_maps, meta


def _run(inputs, trace=False):
    in_maps, meta = _host_prep(
        inputs["x"], inputs["edge_index"], inputs["W1"], inputs["b1"],
        inputs["W2"], inputs["b2"],
    )
    key = (meta["w_list"], meta["xw"])
    if key not in _CACHE:
        _CACHE[key] = _build_program(list(meta["w_list"]), meta["xw"])
    nc = _CACHE[key]
    res = run_bass_kernel_spmd(nc, in_maps, core_ids=list(range(NC)), trace=trace)

    out = np.full((N_NODES, D), -np.inf, np.float32)
    for c in range(NC):
        mc = meta["metas"][c]
        rr = res.results[c]
        r = np.concatenate([rr[f"out{s}"] for s in range(meta["xw"] // MM)], axis=1)
        half, pos, col_node = mc["half"], mc["pos"], mc["col_node"]
        vals = np.empty((mc["ncol"], D), np.float32)
        h0 = half == 0
        vals[h0] = r[0:D, :][:, pos[h0]].T
        vals[~h0] = r[D:P, :][:, pos[~h0]].T
        # merge fold columns per node (max; tanh is monotone)
        nodes = c * NPC + col_node
        np.maximum.at(out, nodes, vals)
        out[c * NPC + np.arange(NPC)[mc["deg"] == 0]] = 0.0
    out[~np.isfinite(out)] = 0.0
    return out, res


def kernel(**inputs) -> np.ndarray:
    out, _ = _run(inputs, trace=False)
    return out
